# revision 1
# baseline (speedup 1.0000x reference)
"""Causal self-attention (GQA + rotary + qk-rmsnorm) on 8 TRN2 NeuronCores.

Sharding: tensor-parallel over (batch, kv-group).  Core c handles batch
b = c//4 and kv-group g = c%4 (4 q heads + 1 kv head), computing the
partial output  y_g @ Wo[256g:256(g+1), :].  The host sums the 4 group
partials per batch (the "all-reduce after c_proj" done at unshard time).

Device layout: everything is computed transposed (T on the free dim).
 - x is pre-transposed host-side: xT (C=1024, T=2048).
 - q projection writes two split-half tiles qA/qB (128, T):
   rows = [h0 dims 0:32 | h1 | h2 | h3] (A = first rotary half, B = second),
   via host-permuted Wq columns.  This keeps rotary fully partition-aligned.
 - k + v projections pack into one tile kvv (128, T):
   rows 0:32 kA, 32:64 kB, 64:128 = v^T (dims 0:64).
 - logits are computed transposed (s on partitions) so the PV matmul needs
   no transposes; the softmax denominator comes from a ones-column in V;
   the k-side rmsnorm scale and the 1/sqrt(64) attention scale are fused
   into exp's per-partition scale operand; the q-side rmsnorm scale is
   applied via a PE outer-product broadcast.
 - no max-subtraction in softmax: q,k are rms-normalized so |logit| <= 8.
All matmuls run in float32r (full PE rate at N>=256, ~1e-4 precision).
"""
import sys
import types

sys.path.insert(0, "/opt/trn_rl_repo")
sys.path.insert(0, "/root/problem")

import math
import numpy as np

# ---------------------------------------------------------------------------
# walrus compat layer (inlined): this environment's walrus build accepts at
# most ONE sync-wait command per instruction, and the Tile tail barrier's
# Drain(eq-wait + update) instructions don't encode.  Patch 1 hoists extra
# waits onto standalone InstEventSemaphore instructions; patch 2 replaces the
# tail with a ge-only drain + count-up barrier + semaphore clears.
# ---------------------------------------------------------------------------

import concourse.bass as bass
import concourse.mybir as mybir
import concourse.tile as tile_mod
from concourse.vector_clock import ScopedClock

_installed = False


def _mk_es(nc, engine, waits=(), updates=()):
    es = mybir.InstEventSemaphore(name=nc.get_next_instruction_name(), ins=[], outs=[])
    es.engine = engine
    es.sync_info = mybir.SyncInfo(on_wait=list(waits), on_update=list(updates))
    return es


def _legalize_lists(nc, ordered):
    n_hoisted = 0
    for bb_name, insts in ordered.items():
        new = []
        for inst in insts:
            si = inst.sync_info
            ws = list(si.on_wait) if si and si.on_wait else []
            if len(ws) > 1:
                # hoist all but the last wait to standalone ESes before inst
                for w in ws[:-1]:
                    es = _mk_es(nc, inst.engine, waits=[w])
                    nc.register_instruction(es, overwrite=True)
                    new.append(es)
                    n_hoisted += 1
                while len(si.on_wait) > 1:
                    si.on_wait.pop(0)
            new.append(inst)
        insts[:] = new
    return n_hoisted


def install():
    global _installed
    if _installed:
        return
    _installed = True

    orig_lower = tile_mod.TileContext._lower_ordered_insts

    def patched_lower(self, ordered):
        _legalize_lists(self.nc, ordered)
        return orig_lower(self, ordered)

    tile_mod.TileContext._lower_ordered_insts = patched_lower

    def patched_drain_and_barrier(self, tick_clock, wait_clock):
        nc = self.nc
        # 1. Collect the global-clock waits on a probe instruction, then
        #    re-emit them as standalone single-wait ESes on SP.
        probe = mybir.InstEventSemaphore(name="wait-probe-unused", ins=[], outs=[])
        probe.engine = mybir.EngineType.SP
        wait_clock.add_sem_waits(probe, ScopedClock({None: tick_clock.global_clock}))
        ws = list(probe.sync_info.on_wait) if probe.sync_info and probe.sync_info.on_wait else []
        sp = nc.engines[mybir.EngineType.SP]
        for w in ws:
            sp.add_instruction(_mk_es(nc, mybir.EngineType.SP, waits=[w]))

        # 2. Bare drains + count-up barrier on a fresh semaphore.
        bar = nc.alloc_semaphore(f"tail_barrier_{nc.next_id()}")
        n_eng = 0
        for eng_type, eng in nc.engines.items():
            d = mybir.InstDrain(
                name=nc.get_next_instruction_name(), ins=[], outs=[],
                bass_is_fusable=False,
            )
            d.engine = eng_type
            eng.add_instruction(d)
            upd = mybir.SyncUpdate(
                sync_type="semaphore", id=bar.num,
                update_mode="sem-inc", update_value=1,
            )
            eng.add_instruction(_mk_es(nc, eng_type, updates=[upd]))
            n_eng += 1

        # 3. gpsimd waits for everyone, clears tile sems + barrier sem.
        pool_eng = mybir.EngineType.Pool
        gw = mybir.SyncWait(
            sync_type="semaphore", id=bar.num,
            wait_mode="sem-ge-imm", wait_value=n_eng,
        )
        nc.engines[pool_eng].add_instruction(_mk_es(nc, pool_eng, waits=[gw]))

        popped = nc._tile_sem_poison_stack.pop()
        assert popped is self._sem_poison
        assert self.sems is not None
        tile_sems = list(self.sems.allocated().values())
        nc.clear_and_free_semaphores(tile_sems)
        nc.gpsimd.sem_clear(range(bar.num, bar.num + 1))

    tile_mod.TileContext._drain_and_barrier = patched_drain_and_barrier


def lint_module(nc, max_waits=1, max_updates=2):
    """Scan final BIR for instructions violating the 1-wait constraint."""
    bad = []
    for f in nc.m.functions:
        for b in f.blocks:
            for inst in b.instructions:
                si = inst.sync_info
                if not si:
                    continue
                nw = len(si.on_wait or [])
                nu = len(si.on_update or [])
                if nw > max_waits or nu > max_updates:
                    bad.append((b.name, type(inst).__name__, inst.name, nw, nu))
    return bad

install()

import concourse.bass as bass
import concourse.mybir as mybir
import concourse.tile as tile
from concourse.bass_utils import run_bass_kernel_spmd
from concourse.masks import make_identity

F32 = mybir.dt.float32
F32R = mybir.dt.float32r
BF16 = mybir.dt.bfloat16
AF = mybir.ActivationFunctionType
ALU = mybir.AluOpType

B, T, C = 2, 2048, 1024
N_HEAD, N_KV, HD = 16, 4, 64
HH = HD // 2  # 32, rotary half
NT = T // 512  # 4 chunks of 512
EPS = 1e-6


def build_nc():
    nc = bass.Bass()
    xT = nc.dram_tensor("xT", (C, T), F32R, kind="ExternalInput")
    cosT4 = nc.dram_tensor("cosT4", (128, T), F32, kind="ExternalInput")
    sinT4 = nc.dram_tensor("sinT4", (128, T), F32, kind="ExternalInput")
    WqA = nc.dram_tensor("WqA", (C, 128), F32R, kind="ExternalInput")
    WqB = nc.dram_tensor("WqB", (C, 128), F32R, kind="ExternalInput")
    Wkvv = nc.dram_tensor("Wkvv", (C, 128), F32R, kind="ExternalInput")
    Wo2 = nc.dram_tensor("Wo2", (256, C), F32R, kind="ExternalInput")
    outp = nc.dram_tensor("outp", (T, C), F32, kind="ExternalOutput")

    with tile.TileContext(nc) as tc:
        with tc.tile_pool(name="persist", bufs=1) as pp, \
             tc.tile_pool(name="mid", bufs=1) as mp:
            # ---- constants ----
            identf = pp.tile([128, 128], F32)
            make_identity(nc, identf[:])
            onesf = pp.tile([128, 1], F32)
            nc.gpsimd.memset(onesf[:], 1.0)
            onesr = pp.tile([128, 1], F32R)
            nc.vector.tensor_copy(onesr[:], onesf[:])
            onesrowf = pp.tile([1, 64], F32)
            nc.gpsimd.memset(onesrowf[:], 1.0)
            onesrow = pp.tile([1, 64], F32R)
            nc.vector.tensor_copy(onesrow[:], onesrowf[:])
            # E4 (128,4): E4[p,m] = 1 iff p//32 == m   (ms head-sum lhsT)
            e4f = pp.tile([128, 4], F32)
            nc.gpsimd.memset(e4f[:], 1.0)
            nc.gpsimd.affine_select(out=e4f[:], in_=e4f[:], compare_op=ALU.is_ge,
                                    fill=0.0, base=0, pattern=[[-32, 4]],
                                    channel_multiplier=1)
            nc.gpsimd.affine_select(out=e4f[:], in_=e4f[:], compare_op=ALU.is_ge,
                                    fill=0.0, base=31, pattern=[[32, 4]],
                                    channel_multiplier=-1)
            e4r = pp.tile([128, 4], F32R)
            nc.vector.tensor_copy(e4r[:], e4f[:])
            # SEL (4,128): SEL[k,j] = 1 iff j//32 == k   (w_q broadcast lhsT)
            self_ = pp.tile([4, 128], F32)
            nc.gpsimd.memset(self_[:], 1.0)
            nc.gpsimd.affine_select(out=self_[:], in_=self_[:], compare_op=ALU.is_ge,
                                    fill=0.0, base=0, pattern=[[1, 128]],
                                    channel_multiplier=-32)
            nc.gpsimd.affine_select(out=self_[:], in_=self_[:], compare_op=ALU.is_ge,
                                    fill=0.0, base=31, pattern=[[-1, 128]],
                                    channel_multiplier=32)
            selr = pp.tile([4, 128], F32R)
            nc.vector.tensor_copy(selr[:], self_[:])
            eps4 = pp.tile([4, 1], F32)
            nc.gpsimd.memset(eps4[:], EPS)
            eps128 = pp.tile([128, 1], F32)
            nc.gpsimd.memset(eps128[:], EPS)
            bln8 = pp.tile([128, 1], F32)
            nc.gpsimd.memset(bln8[:], -math.log(8.0))
            identr = pp.tile([128, 128], F32R)
            nc.vector.tensor_copy(identr[:], identf[:])
            identb = pp.tile([128, 128], BF16)
            nc.vector.tensor_copy(identb[:], identf[:])

            # ---- persistent attention operands ----
            qpairc = [[pp.tile([128, 512], BF16, name=f"qpair{m}_{c}", tag=f"qpair{m}_{c}")
                       for c in range(NT)] for m in range(2)]
            kT2c = [pp.tile([128, 512], BF16, name=f"kT2_{c}", tag=f"kT2_{c}")
                    for c in range(NT)]
            v_aug = [pp.tile([128, HD + 1], BF16, name=f"vaug{i}", tag=f"vaug{i}") for i in range(16)]
            u8c = [pp.tile([128, 4], F32, name=f"u8c{c}", tag=f"u8c{c}") for c in range(NT)]
            yhat = [pp.tile([128, T], F32R, name=f"yhat{m}", tag=f"yhat{m}") for m in range(2)]

            # mid-scoped (freed before attention)
            qAraw = [mp.tile([128, 512], F32, name=f"qAraw{c}", tag=f"qAraw{c}") for c in range(NT)]
            qBraw = [mp.tile([128, 512], F32, name=f"qBraw{c}", tag=f"qBraw{c}") for c in range(NT)]
            kvvraw = [mp.tile([128, 512], F32, name=f"kvvraw{c}", tag=f"kvvraw{c}") for c in range(NT)]
            cos_sb = mp.tile([128, T], F32)
            sin_sb = mp.tile([128, T], F32)
            nc.sync.dma_start(cos_sb[:], cosT4[:])
            nc.sync.dma_start(sin_sb[:], sinT4[:])

            # =========== P0: projections ===========
            with tc.tile_pool(name="p0sb", bufs=1) as p0, \
                 tc.tile_pool(name="p0ps", bufs=4, space="PSUM") as pps:
                xT_sb = [p0.tile([128, T], F32R, name=f"xt{c}", tag=f"xt{c}") for c in range(8)]
                for c in range(8):
                    nc.sync.dma_start(xT_sb[c][:], xT[128 * c:128 * (c + 1), :])
                wA_sb = [p0.tile([128, 128], F32R, name=f"wa{c}", tag=f"wa{c}") for c in range(8)]
                wB_sb = [p0.tile([128, 128], F32R, name=f"wb{c}", tag=f"wb{c}") for c in range(8)]
                wK_sb = [p0.tile([128, 128], F32R, name=f"wk{c}", tag=f"wk{c}") for c in range(8)]
                for c in range(8):
                    nc.sync.dma_start(wA_sb[c][:], WqA[128 * c:128 * (c + 1), :])
                    nc.sync.dma_start(wB_sb[c][:], WqB[128 * c:128 * (c + 1), :])
                    nc.sync.dma_start(wK_sb[c][:], Wkvv[128 * c:128 * (c + 1), :])
                for t4 in range(NT):
                    for dst, wsb in ((qAraw, wA_sb), (qBraw, wB_sb), (kvvraw, wK_sb)):
                        ps = pps.tile([128, 512], F32, tag="proj")
                        for c in range(8):
                            nc.tensor.matmul(
                                ps[:], wsb[c][:], xT_sb[c][:, 512 * t4:512 * (t4 + 1)],
                                start=(c == 0), stop=(c == 7))
                        nc.scalar.copy(dst[t4][:], ps[:])

            # =========== P1: rmsnorm stats, rotary, v transpose ===========
            with tc.tile_pool(name="p1sb", bufs=1) as p1, \
                 tc.tile_pool(name="p1sq", bufs=2) as p1sq, \
                 tc.tile_pool(name="p1ps", bufs=2, space="PSUM") as msps, \
                 tc.tile_pool(name="p1bc", bufs=2, space="PSUM") as bcps, \
                 tc.tile_pool(name="p1tp", bufs=2, space="PSUM") as tpps:
                msq_sb = [p1.tile([4, 512], F32, name=f"msq{c}", tag=f"msq{c}") for c in range(NT)]
                wqr = [p1.tile([4, 512], F32R, name=f"wqr{c}", tag=f"wqr{c}") for c in range(NT)]
                for t4 in range(NT):
                    sl = slice(512 * t4, 512 * (t4 + 1))
                    sqA = p1sq.tile([128, 512], F32R, tag="sq")
                    nc.gpsimd.tensor_mul(sqA[:], qAraw[t4][:], qAraw[t4][:])
                    sqB = p1sq.tile([128, 512], F32R, tag="sq")
                    nc.gpsimd.tensor_mul(sqB[:], qBraw[t4][:], qBraw[t4][:])
                    mq = msps.tile([4, 512], F32, tag="ms")
                    nc.tensor.matmul(mq[:], e4r[:], sqA[:], start=True, stop=False)
                    nc.tensor.matmul(mq[:], e4r[:], sqB[:], start=False, stop=True)
                    nc.scalar.activation(msq_sb[t4][:], mq[:], AF.Ln, bias=eps4[:], scale=1.0 / HD)
                    nc.scalar.activation(wqr[t4][:], msq_sb[t4][:], AF.Exp, bias=0.0, scale=-0.5)

                # rotary + q normalization, written directly into qpair
                for t4 in range(NT):
                    sl = slice(512 * t4, 512 * (t4 + 1))
                    bc = bcps.tile([128, 512], F32, tag="bc")
                    nc.tensor.matmul(bc[:], selr[:], wqr[t4][:], start=True, stop=True)
                    tq = p1sq.tile([128, 512], F32, tag="rot")
                    tq2 = p1sq.tile([128, 512], F32, tag="rot")
                    # y1 = (qA*cos + qB*sin) * w ; y2 = (qB*cos - qA*sin) * w
                    nc.vector.tensor_mul(tq[:], qAraw[t4][:], cos_sb[:, sl])
                    nc.vector.tensor_mul(tq2[:], qBraw[t4][:], sin_sb[:, sl])
                    nc.vector.tensor_add(tq[:], tq[:], tq2[:])
                    for hq in range(4):
                        nc.vector.tensor_mul(
                            qpairc[hq // 2][t4][64 * (hq % 2):64 * (hq % 2) + 32, :],
                            tq[32 * hq:32 * (hq + 1), :],
                            bc[32 * hq:32 * (hq + 1), :])
                    nc.vector.tensor_mul(tq[:], qBraw[t4][:], cos_sb[:, sl])
                    nc.vector.tensor_mul(tq2[:], qAraw[t4][:], sin_sb[:, sl])
                    nc.vector.tensor_sub(tq[:], tq[:], tq2[:])
                    for hq in range(4):
                        nc.vector.tensor_mul(
                            qpairc[hq // 2][t4][64 * (hq % 2) + 32:64 * (hq % 2) + 64, :],
                            tq[32 * hq:32 * (hq + 1), :],
                            bc[32 * hq:32 * (hq + 1), :])
                    # k rotary (head 4g): rows 0:32 / 32:64 of kvvraw
                    tk = p1sq.tile([32, 512], F32, tag="rotk")
                    tk2 = p1sq.tile([32, 512], F32, tag="rotk")
                    nc.gpsimd.tensor_mul(tk[:], kvvraw[t4][0:32, :], cos_sb[0:32, sl])
                    nc.gpsimd.tensor_mul(tk2[:], kvvraw[t4][32:64, :], sin_sb[32:64, sl])
                    nc.gpsimd.tensor_add(kT2c[t4][0:32, :], tk[:], tk2[:])
                    nc.gpsimd.tensor_mul(tk[:], kvvraw[t4][32:64, :], cos_sb[32:64, sl])
                    nc.gpsimd.tensor_mul(tk2[:], kvvraw[t4][0:32, :], sin_sb[0:32, sl])
                    nc.gpsimd.tensor_sub(kT2c[t4][32:64, :], tk[:], tk2[:])
                    nc.gpsimd.tensor_copy(kT2c[t4][64:128, :], kT2c[t4][0:64, :])
                # k-side norm scale, in s-column layout, per chunk
                for cku in range(NT):
                    u8f = p1sq.tile([128, 4], F32, tag="u8f")
                    for lt in range(4):
                        tt = 4 * cku + lt
                        ktp = tpps.tile([128, 64], BF16, tag="ktp")
                        nc.tensor.transpose(
                            ktp[:], kT2c[cku][0:64, 128 * lt:128 * (lt + 1)],
                            identb[0:64, 0:64])
                        knat = p1sq.tile([128, 64], F32, tag="knat")
                        nc.scalar.copy(knat[:], ktp[:])
                        ksq = p1sq.tile([128, 64], F32, tag="ksq")
                        nc.vector.tensor_mul(ksq[:], knat[:], ktp[:])
                        nc.vector.reduce_sum(u8f[:, lt:lt + 1], ksq[:], axis=mybir.AxisListType.X)
                    nc.scalar.activation(u8f[:], u8f[:], AF.Ln, bias=eps128[:], scale=1.0 / HD)
                    nc.scalar.activation(u8c[cku][:], u8f[:], AF.Exp, bias=bln8[:], scale=-0.5)

                # v transpose: kvvraw[64:128] (64, T) -> v_aug tiles (128, 65)
                for tt in range(16):
                    tp = tpps.tile([128, 64], F32, tag="tp")
                    nc.tensor.transpose(
                        tp[:], kvvraw[tt // 4][64:128, 128 * (tt % 4):128 * (tt % 4 + 1)],
                        identf[64:128, 64:128])
                    nc.scalar.copy(v_aug[tt][:, 0:64], tp[:])
                    nc.vector.tensor_copy(v_aug[tt][:, 64:65], onesf[:])

            # =========== P2: attention + fused output projection ===========
            with tc.tile_pool(name="p2wt", bufs=4) as wtp, \
                 tc.tile_pool(name="p2rd", bufs=2) as rdp, \
                 tc.tile_pool(name="p2o", bufs=3) as outsb, \
                 tc.tile_pool(name="lgps", bufs=4, space="PSUM") as lgps, \
                 tc.tile_pool(name="ytps", bufs=2, space="PSUM") as ytps, \
                 tc.tile_pool(name="bcps2", bufs=2, space="PSUM") as bcps2:
                Wo_sb = [outsb.tile([128, C], F32R, name=f"wo{m}", tag=f"wo{m}")
                         for m in range(2)]
                for m in range(2):
                    nc.sync.dma_start(Wo_sb[m][:], Wo2[128 * m:128 * (m + 1), :])
                for c4 in range(4):
                    t0 = 512 * c4
                    n_st = 4 * c4 + 4
                    for pidx in range(2):
                        py = [ytps.tile([65, 512], F32, tag="yt",
                                        name=f"py_{pidx}_{c4}_{hh}")
                              for hh in range(2)]

                        def emit_qk(st):
                            vs = max(0, 128 * st - t0)
                            qs = 0 if vs < 256 else 256
                            lgs = []
                            for hh in range(2):
                                lg = lgps.tile([128, 512], F32, tag="lg",
                                               name=f"lg_{pidx}_{c4}_{st}_{hh}")
                                nc.tensor.matmul(
                                    lg[:, qs:512],
                                    kT2c[st // 4][64 * hh:64 * (hh + 1),
                                                  128 * (st % 4):128 * (st % 4 + 1)],
                                    qpairc[pidx][c4][64 * hh:64 * (hh + 1), qs:512],
                                    start=True, stop=True,
                                    tile_position=(64 * hh, 0))
                                lgs.append(lg)
                            return lgs

                        def emit_exp_pv(st, lgs):
                            vs = max(0, 128 * st - t0)
                            qs = 0 if vs < 256 else 256
                            b0 = t0 + qs - 128 * st
                            for hh in range(2):
                                wt = wtp.tile([128, 512], BF16, tag="wt",
                                              name=f"wt_{pidx}_{c4}_{st}_{hh}")
                                nc.scalar.activation(
                                    wt[:, qs:512], lgs[hh][:, qs:512], AF.Exp,
                                    bias=0.0, scale=u8c[st // 4][:, st % 4:st % 4 + 1])
                                if b0 < 127:
                                    nc.gpsimd.affine_select(
                                        out=wt[:, qs:512], in_=wt[:, qs:512],
                                        compare_op=ALU.is_ge, fill=0.0, base=b0,
                                        pattern=[[1, 512 - qs]],
                                        channel_multiplier=-1)
                                nc.tensor.matmul(
                                    py[hh][:, qs:512], v_aug[st][:],
                                    wt[:, qs:512],
                                    start=(st == 0), stop=(st == n_st - 1))

                        prev = None
                        for st in range(n_st):
                            lgs = emit_qk(st)
                            if prev is not None:
                                emit_exp_pv(st - 1, prev)
                            prev = lgs
                        emit_exp_pv(n_st - 1, prev)

                        for hh in range(2):
                            # drain py fast (frees the yt slot); the slow
                            # recip/broadcast chain runs off the critical path
                            drow = rdp.tile([1, 512], F32, tag="drow")
                            nc.vector.tensor_copy(drow[:], py[hh][64:65, :])
                            ysb = rdp.tile([64, 512], F32, tag="ysb")
                            nc.vector.tensor_copy(ysb[:], py[hh][0:64, :])
                            rd = rdp.tile([1, 512], F32, tag="rd")
                            nc.vector.reciprocal(rd[:], drow[:])
                            rdr = rdp.tile([1, 512], F32R, tag="rdr")
                            nc.vector.tensor_copy(rdr[:], rd[:])
                            bc2 = bcps2.tile([64, 512], F32, tag="bc2")
                            nc.tensor.matmul(bc2[:], onesrow[:], rdr[:],
                                             start=True, stop=True)
                            r0 = 64 * hh
                            nc.vector.tensor_mul(
                                yhat[pidx][r0:r0 + 64, t0:t0 + 512],
                                ysb[:], bc2[:])
                    # output projection for this chunk's four t-tiles
                    for tt in range(4 * c4, 4 * c4 + 4):
                        for ch in range(2):
                            po = bcps2.tile([128, 512], F32, tag="bc2",
                                            name=f"po_{tt}_{ch}")
                            for m in range(2):
                                nc.tensor.matmul(
                                    po[:], yhat[m][:, 128 * tt:128 * (tt + 1)],
                                    Wo_sb[m][:, 512 * ch:512 * (ch + 1)],
                                    start=(m == 0), stop=(m == 1))
                            osb = outsb.tile([128, 512], F32, tag="osb")
                            nc.scalar.copy(osb[:], po[:])
                            nc.sync.dma_start(
                                outp[128 * tt:128 * (tt + 1), 512 * ch:512 * (ch + 1)],
                                osb[:])
    return nc


_nc_cache = None


def _get_nc():
    global _nc_cache
    if _nc_cache is None:
        _nc_cache = build_nc()
    return _nc_cache


def make_in_maps(x, cos, sin, Wq, Wk, Wv, Wo):
    cosT = np.ascontiguousarray(cos[0, :, 0, :].T)   # (32, T)
    sinT = np.ascontiguousarray(sin[0, :, 0, :].T)
    cosT4 = np.ascontiguousarray(np.tile(cosT, (4, 1)))  # (128, T)
    sinT4 = np.ascontiguousarray(np.tile(sinT, (4, 1)))
    in_maps = []
    for c in range(8):
        b, g = c // 4, c % 4
        heads = [4 * g + i for i in range(4)]
        permA = [64 * h + d for h in heads for d in range(HH)]
        permB = [64 * h + HH + d for h in heads for d in range(HH)]
        WqA = np.ascontiguousarray(Wq[:, permA])
        WqB = np.ascontiguousarray(Wq[:, permB])
        Wkvv = np.ascontiguousarray(np.concatenate(
            [Wk[:, 64 * g:64 * g + HH], Wk[:, 64 * g + HH:64 * (g + 1)],
             Wv[:, 64 * g:64 * (g + 1)]], axis=1))
        Wo2 = np.ascontiguousarray(
            Wo[256 * g:256 * (g + 1), :].reshape(2, 128, C))
        xT = np.ascontiguousarray(x[b].T)
        in_maps.append({
            "xT": xT, "cosT4": cosT4, "sinT4": sinT4,
            "WqA": WqA, "WqB": WqB, "Wkvv": Wkvv, "Wo2": Wo2,
        })
    return in_maps


def run(inputs, trace=False, **kwargs):
    nc = _get_nc()
    in_maps = make_in_maps(**inputs)
    res = run_bass_kernel_spmd(nc, in_maps, core_ids=list(range(8)),
                               trace=trace, **kwargs)
    outs = [res.results[c]["outp"] for c in range(8)]
    full = np.stack([
        outs[0] + outs[1] + outs[2] + outs[3],
        outs[4] + outs[5] + outs[6] + outs[7],
    ]).astype(np.float32)
    return full, res


def kernel(**inputs):
    out, _ = run({k: np.asarray(v) for k, v in inputs.items()}, trace=False)
    return out

